# revision 11
# baseline (speedup 1.0000x reference)
"""TRN2 Bass kernel for nn_ConceptEmbeddingConceptPred.

Computes y = concat([einsum('bjd,ijd->bi', x, W_emb) + b_loo,
                     einsum('bjd,hjd->bh', x, W_full) + b_full], axis=1)
where W_emb is the leave-one-out scatter-embedding of W_loo (zero diagonal).

Flattened, this is a (4096 x 16384) @ (16384 x 136) GEMM.

Distribution: contraction(k)-parallel over the 8 cores — core c owns
concepts j in [16c, 16c+16) (k-slice of 2048). Each core computes a full
(136, 4096) partial product; partials are summed on the host (cheap:
8 x 2.2 MB), bias added, transposed, concatenated.

Per-core dataflow (fp32r = hardware fast-fp32, ~1.5e-4 rel err):
  - x arrives natural-layout (b, k); contraction must sit on SBUF
    partitions, so each 128x128 block is transposed on the tensor engine
    (fp32r transpose mode, ~77ns/tile measured) via an identity matmul,
    staged through PSUM, copied to SBUF by DVE/ACT.
  - loo matmul: stationary = W_embT k-tile (128x128), moving = xT
    (128x512) accumulating over 16 k-tiles into one PSUM bank.
  - full-probe matmul (M=8): plain accumulating matmuls at partition
    base 0 (this walrus rejects fp32r matmuls with dst partition base
    != 0, so 32-col-group packing via tile_position is unavailable).
"""

import sys

for _p in ("/opt/trn_rl_repo",):
    if _p not in sys.path:
        sys.path.append(_p)

import numpy as np
import concourse.bacc as bacc
import concourse.mybir as mybir
import concourse.tile as tile
from concourse.bass_utils import run_bass_kernel_spmd

dt = mybir.dt

B, C, D, H = 4096, 128, 128, 8
NCORES = 8
JPC = C // NCORES  # 16 concept (= k) tiles per core
KPC = JPC * D  # 2048 contraction elements per core
BCHUNK = 512  # batch per PSUM accumulation chunk (fp32 bank limit)
NBC = B // BCHUNK  # 8 batch chunks
NBT = BCHUNK // 128  # 4 b-tiles of 128 per chunk

_nc_cache = None


def _build():
    global _nc_cache
    if _nc_cache is not None:
        return _nc_cache

    nc = bacc.Bacc(
        "TRN2", target_bir_lowering=False, debug=False, num_devices=NCORES
    )
    x_d = nc.dram_tensor("x", (B, KPC), dt.float32r, kind="ExternalInput").ap()
    wl_d = nc.dram_tensor(
        "w_loo_t", (JPC, D, C), dt.float32r, kind="ExternalInput"
    ).ap()
    wf_d = nc.dram_tensor(
        "w_full_t", (JPC, D, H), dt.float32r, kind="ExternalInput"
    ).ap()
    id_d = nc.dram_tensor("ident", (128, 128), dt.float32r, kind="ExternalInput").ap()
    yl_d = nc.dram_tensor("y_loo_t", (C, B), dt.float32, kind="ExternalOutput").ap()
    yf_d = nc.dram_tensor("y_full_t", (H, B), dt.float32, kind="ExternalOutput").ap()

    with tile.TileContext(nc) as tc:
        with (
            tc.tile_pool(name="wpool", bufs=1) as wpool,
            tc.tile_pool(name="xpool", bufs=14) as xpool,
            tc.tile_pool(name="xtpool", bufs=8) as xtpool,
            tc.tile_pool(name="ypool", bufs=2) as ypool,
            tc.tile_pool(name="pst", bufs=3, space="PSUM") as pst,
            tc.tile_pool(name="psl", bufs=2, space="PSUM") as psl,
            tc.tile_pool(name="psf", bufs=2, space="PSUM") as psf,
        ):
            wl = wpool.tile([D, JPC, C], dt.float32r)
            wf = wpool.tile([D, JPC, H], dt.float32r)
            ident = wpool.tile([128, 128], dt.float32r)
            nc.sync.dma_start(ident[:], id_d[:])

            for bc in range(NBC):
                xns = []
                for bt in range(NBT):
                    xn = xpool.tile([128, KPC], dt.float32r, tag="xn")
                    xns.append(xn)
                if bc == 0:
                    # k-chunked, bt-interleaved loads so the first transposes
                    # can start after ~1MB instead of ~4MB
                    ck = 512
                    for c0 in range(0, KPC, ck):
                        for bt in range(NBT):
                            r0 = bt * 128
                            nc.sync.dma_start(
                                xns[bt][:, c0 : c0 + ck],
                                x_d[r0 : r0 + 128, c0 : c0 + ck],
                            )
                        if c0 == 0:
                            nc.sync.dma_start(wf[:], wf_d.rearrange("t d h -> d t h"))
                            nc.sync.dma_start(wl[:], wl_d.rearrange("t d i -> d t i"))
                else:
                    for bt in range(NBT):
                        r0 = (bc * NBT + bt) * 128
                        hk = KPC // 2
                        # split across HWDGE (sync) and SWDGE (gpsimd) queue
                        # families for more DMA queue parallelism
                        eng = nc.sync if bt % 2 == 0 else nc.gpsimd
                        eng.dma_start(xns[bt][:, :hk], x_d[r0 : r0 + 128, :hk])
                        eng.dma_start(xns[bt][:, hk:], x_d[r0 : r0 + 128, hk:])

                acc_l = psl.tile([C, BCHUNK], dt.float32, tag="accl")
                acc_f = psf.tile([H, BCHUNK], dt.float32, tag="accf")
                for kt in range(JPC):
                    ps_xt = pst.tile([128, BCHUNK], dt.float32r, tag="pst")
                    for bt in range(NBT):
                        nc.tensor.transpose(
                            ps_xt[:, bt * 128 : (bt + 1) * 128],
                            xns[bt][:, kt * 128 : (kt + 1) * 128],
                            ident[:],
                        )
                    xt = xtpool.tile([128, BCHUNK], dt.float32r, tag="xt")
                    # split PSUM->SBUF copy load between DVE and ACT
                    if kt % 2 == 0:
                        nc.vector.tensor_copy(xt[:], ps_xt[:])
                    else:
                        nc.scalar.copy(xt[:], ps_xt[:])

                    # full first: its stationary is 8 cols (trivial LDW),
                    # and the loo matmul's 224ns weight load can then hide
                    # under the full matmul's 512-col stream
                    nc.tensor.matmul(
                        acc_f[:],
                        wf[:, kt, :],
                        xt[:],
                        start=(kt == 0),
                        stop=(kt == JPC - 1),
                    )
                    nc.tensor.matmul(
                        acc_l[:],
                        wl[:, kt, :],
                        xt[:],
                        start=(kt == 0),
                        stop=(kt == JPC - 1),
                    )

                yl_sb = ypool.tile([C, BCHUNK], dt.float32, tag="yl")
                nc.vector.tensor_copy(yl_sb[:], acc_l[:])
                nc.sync.dma_start(yl_d[:, bc * BCHUNK : (bc + 1) * BCHUNK], yl_sb[:])

                yf_sb = ypool.tile([H, BCHUNK], dt.float32, tag="yf")
                nc.vector.tensor_copy(yf_sb[:], acc_f[:])
                nc.sync.dma_start(yf_d[:, bc * BCHUNK : (bc + 1) * BCHUNK], yf_sb[:])

    nc.compile()
    _nc_cache = nc
    return nc


def _embed_loo_weights(W_loo):
    # probe i sees concepts j != i; scatter into (C, C, D) with zero row at j=i
    I = np.arange(C)[:, None]
    J = np.arange(C)[None, :]
    src = np.clip(J - (J > I).astype(np.int64), 0, C - 2)  # (C, C)
    W_emb = np.take_along_axis(W_loo, src[:, :, None], axis=1)  # (C, C, D)
    return W_emb * (J != I)[:, :, None].astype(W_loo.dtype)


def _prep_in_maps(x, W_loo, W_full):
    x = np.asarray(x, dtype=np.float32)
    W_emb = _embed_loo_weights(np.asarray(W_loo, dtype=np.float32))
    W_full = np.asarray(W_full, dtype=np.float32)
    ident = np.eye(128, dtype=np.float32)
    in_maps = []
    for c in range(NCORES):
        jsl = slice(c * JPC, (c + 1) * JPC)
        x_c = np.ascontiguousarray(x[:, jsl, :]).reshape(B, KPC)
        wl_c = np.ascontiguousarray(W_emb[:, jsl, :].transpose(1, 2, 0))  # (t,d,i)
        wf_c = np.ascontiguousarray(W_full[:, jsl, :].transpose(1, 2, 0))  # (t,d,h)
        in_maps.append(
            {"x": x_c, "w_loo_t": wl_c, "w_full_t": wf_c, "ident": ident}
        )
    return in_maps


def _assemble(results, b_loo, b_full):
    y_loo_t = np.zeros((C, B), np.float64)
    y_full_t = np.zeros((H, B), np.float64)
    for r in results:
        y_loo_t += r["y_loo_t"]
        y_full_t += r["y_full_t"]
    y_loo = (y_loo_t.T + np.asarray(b_loo, np.float64)[None, :]).astype(np.float32)
    y_full = (y_full_t.T + np.asarray(b_full, np.float64)[None, :]).astype(np.float32)
    return np.concatenate([y_loo, y_full], axis=1)


def run_spmd(x, W_loo, b_loo, W_full, b_full, trace=False):
    nc = _build()
    in_maps = _prep_in_maps(x, W_loo, W_full)
    res = run_bass_kernel_spmd(
        nc, in_maps, core_ids=list(range(NCORES)), trace=trace
    )
    return _assemble(res.results, b_loo, b_full), res


def kernel(x, W_loo, b_loo, W_full, b_full):
    out, _ = run_spmd(x, W_loo, b_loo, W_full, b_full)
    return out


# revision 12
# speedup vs baseline: 1.0416x; 1.0416x over previous
"""TRN2 Bass kernel for nn_ConceptEmbeddingConceptPred.

Computes y = concat([einsum('bjd,ijd->bi', x, W_emb) + b_loo,
                     einsum('bjd,hjd->bh', x, W_full) + b_full], axis=1)
where W_emb is the leave-one-out scatter-embedding of W_loo (zero diagonal).

Flattened, this is a (4096 x 16384) @ (16384 x 136) GEMM.

Distribution: contraction(k)-parallel over the 8 cores — core c owns
concepts j in [16c, 16c+16) (k-slice of 2048). Each core computes a full
(136, 4096) partial product; partials are summed on the host (cheap:
8 x 2.2 MB), bias added, transposed, concatenated.

Per-core dataflow (fp32r = hardware fast-fp32, ~1.5e-4 rel err):
  - x arrives natural-layout (b, k); contraction must sit on SBUF
    partitions, so each 128x128 block is transposed on the tensor engine
    (fp32r transpose mode, ~77ns/tile measured) via an identity matmul,
    staged through PSUM, copied to SBUF by DVE/ACT.
  - loo matmul: stationary = W_embT k-tile (128x128), moving = xT
    (128x512) accumulating over 16 k-tiles into one PSUM bank.
  - full-probe matmul (M=8): plain accumulating matmuls at partition
    base 0 (this walrus rejects fp32r matmuls with dst partition base
    != 0, so 32-col-group packing via tile_position is unavailable).
"""

import sys

for _p in ("/opt/trn_rl_repo",):
    if _p not in sys.path:
        sys.path.append(_p)

import numpy as np
import concourse.bacc as bacc
import concourse.mybir as mybir
import concourse.tile as tile
from concourse.bass_utils import run_bass_kernel_spmd

dt = mybir.dt

B, C, D, H = 4096, 128, 128, 8
NCORES = 8
JPC = C // NCORES  # 16 concept (= k) tiles per core
KPC = JPC * D  # 2048 contraction elements per core
BCHUNK = 512  # batch per PSUM accumulation chunk (fp32 bank limit)
NBC = B // BCHUNK  # 8 batch chunks
NBT = BCHUNK // 128  # 4 b-tiles of 128 per chunk

_nc_cache = None


def _build():
    global _nc_cache
    if _nc_cache is not None:
        return _nc_cache

    nc = bacc.Bacc(
        "TRN2", target_bir_lowering=False, debug=False, num_devices=NCORES
    )
    x_d = nc.dram_tensor("x", (B, KPC), dt.float32r, kind="ExternalInput").ap()
    wl_d = nc.dram_tensor(
        "w_loo_t", (JPC, D, C), dt.float32r, kind="ExternalInput"
    ).ap()
    wf_d = nc.dram_tensor(
        "w_full_t", (JPC, D, H), dt.float32r, kind="ExternalInput"
    ).ap()
    id_d = nc.dram_tensor("ident", (128, 128), dt.float32r, kind="ExternalInput").ap()
    yl_d = nc.dram_tensor("y_loo_t", (C, B), dt.float32, kind="ExternalOutput").ap()
    yf_d = nc.dram_tensor("y_full_t", (H, B), dt.float32, kind="ExternalOutput").ap()

    with tile.TileContext(nc) as tc:
        with (
            tc.tile_pool(name="wpool", bufs=1) as wpool,
            tc.tile_pool(name="xpool", bufs=14) as xpool,
            tc.tile_pool(name="xtpool", bufs=8) as xtpool,
            tc.tile_pool(name="ypool", bufs=2) as ypool,
            tc.tile_pool(name="pst", bufs=3, space="PSUM") as pst,
            tc.tile_pool(name="psl", bufs=2, space="PSUM") as psl,
            tc.tile_pool(name="psf", bufs=2, space="PSUM") as psf,
        ):
            wl = wpool.tile([D, JPC, C], dt.float32r)
            wf = wpool.tile([D, JPC, H], dt.float32r)
            ident = wpool.tile([128, 128], dt.float32r)
            nc.sync.dma_start(ident[:], id_d[:])

            for bc in range(NBC):
                xns = []
                for bt in range(NBT):
                    xn = xpool.tile([128, KPC], dt.float32r, tag="xn")
                    xns.append(xn)
                if bc == 0:
                    # k-chunked, bt-interleaved loads so the first transposes
                    # can start after ~1MB instead of ~4MB
                    ck = 512
                    for c0 in range(0, KPC, ck):
                        for bt in range(NBT):
                            r0 = bt * 128
                            nc.sync.dma_start(
                                xns[bt][:, c0 : c0 + ck],
                                x_d[r0 : r0 + 128, c0 : c0 + ck],
                            )
                        if c0 == 0:
                            nc.sync.dma_start(wf[:], wf_d.rearrange("t d h -> d t h"))
                            nc.sync.dma_start(wl[:], wl_d.rearrange("t d i -> d t i"))
                else:
                    for bt in range(NBT):
                        r0 = (bc * NBT + bt) * 128
                        hk = KPC // 2
                        nc.sync.dma_start(xns[bt][:, :hk], x_d[r0 : r0 + 128, :hk])
                        nc.sync.dma_start(xns[bt][:, hk:], x_d[r0 : r0 + 128, hk:])

                acc_l = psl.tile([C, BCHUNK], dt.float32, tag="accl")
                acc_f = psf.tile([H, BCHUNK], dt.float32, tag="accf")
                for kt in range(JPC):
                    ps_xt = pst.tile([128, BCHUNK], dt.float32r, tag="pst")
                    for bt in range(NBT):
                        nc.tensor.transpose(
                            ps_xt[:, bt * 128 : (bt + 1) * 128],
                            xns[bt][:, kt * 128 : (kt + 1) * 128],
                            ident[:],
                        )
                    xt = xtpool.tile([128, BCHUNK], dt.float32r, tag="xt")
                    # split PSUM->SBUF copy load between DVE and ACT
                    if kt % 2 == 0:
                        nc.vector.tensor_copy(xt[:], ps_xt[:])
                    else:
                        nc.scalar.copy(xt[:], ps_xt[:])

                    # full first: its stationary is 8 cols (trivial LDW),
                    # and the loo matmul's 224ns weight load can then hide
                    # under the full matmul's 512-col stream
                    nc.tensor.matmul(
                        acc_f[:],
                        wf[:, kt, :],
                        xt[:],
                        start=(kt == 0),
                        stop=(kt == JPC - 1),
                    )
                    nc.tensor.matmul(
                        acc_l[:],
                        wl[:, kt, :],
                        xt[:],
                        start=(kt == 0),
                        stop=(kt == JPC - 1),
                    )

                yl_sb = ypool.tile([C, BCHUNK], dt.float32, tag="yl")
                nc.vector.tensor_copy(yl_sb[:], acc_l[:])
                nc.sync.dma_start(yl_d[:, bc * BCHUNK : (bc + 1) * BCHUNK], yl_sb[:])

                yf_sb = ypool.tile([H, BCHUNK], dt.float32, tag="yf")
                nc.vector.tensor_copy(yf_sb[:], acc_f[:])
                nc.sync.dma_start(yf_d[:, bc * BCHUNK : (bc + 1) * BCHUNK], yf_sb[:])

    nc.compile()
    _nc_cache = nc
    return nc


def _embed_loo_weights(W_loo):
    # probe i sees concepts j != i; scatter into (C, C, D) with zero row at j=i
    I = np.arange(C)[:, None]
    J = np.arange(C)[None, :]
    src = np.clip(J - (J > I).astype(np.int64), 0, C - 2)  # (C, C)
    W_emb = np.take_along_axis(W_loo, src[:, :, None], axis=1)  # (C, C, D)
    return W_emb * (J != I)[:, :, None].astype(W_loo.dtype)


def _prep_in_maps(x, W_loo, W_full):
    x = np.asarray(x, dtype=np.float32)
    W_emb = _embed_loo_weights(np.asarray(W_loo, dtype=np.float32))
    W_full = np.asarray(W_full, dtype=np.float32)
    ident = np.eye(128, dtype=np.float32)
    in_maps = []
    for c in range(NCORES):
        jsl = slice(c * JPC, (c + 1) * JPC)
        x_c = np.ascontiguousarray(x[:, jsl, :]).reshape(B, KPC)
        wl_c = np.ascontiguousarray(W_emb[:, jsl, :].transpose(1, 2, 0))  # (t,d,i)
        wf_c = np.ascontiguousarray(W_full[:, jsl, :].transpose(1, 2, 0))  # (t,d,h)
        in_maps.append(
            {"x": x_c, "w_loo_t": wl_c, "w_full_t": wf_c, "ident": ident}
        )
    return in_maps


def _assemble(results, b_loo, b_full):
    y_loo_t = np.zeros((C, B), np.float64)
    y_full_t = np.zeros((H, B), np.float64)
    for r in results:
        y_loo_t += r["y_loo_t"]
        y_full_t += r["y_full_t"]
    y_loo = (y_loo_t.T + np.asarray(b_loo, np.float64)[None, :]).astype(np.float32)
    y_full = (y_full_t.T + np.asarray(b_full, np.float64)[None, :]).astype(np.float32)
    return np.concatenate([y_loo, y_full], axis=1)


def run_spmd(x, W_loo, b_loo, W_full, b_full, trace=False):
    nc = _build()
    in_maps = _prep_in_maps(x, W_loo, W_full)
    res = run_bass_kernel_spmd(
        nc, in_maps, core_ids=list(range(NCORES)), trace=trace
    )
    return _assemble(res.results, b_loo, b_full), res


def kernel(x, W_loo, b_loo, W_full, b_full):
    out, _ = run_spmd(x, W_loo, b_loo, W_full, b_full)
    return out


# revision 13
# speedup vs baseline: 1.0501x; 1.0081x over previous
"""TRN2 Bass kernel for nn_ConceptEmbeddingConceptPred.

Computes y = concat([einsum('bjd,ijd->bi', x, W_emb) + b_loo,
                     einsum('bjd,hjd->bh', x, W_full) + b_full], axis=1)
where W_emb is the leave-one-out scatter-embedding of W_loo (zero diagonal).

Flattened, this is a (4096 x 16384) @ (16384 x 136) GEMM.

Distribution: contraction(k)-parallel over the 8 cores — core c owns
concepts j in [16c, 16c+16) (k-slice of 2048). Each core computes a full
(136, 4096) partial product; partials are summed on the host (cheap:
8 x 2.2 MB), bias added, transposed, concatenated.

Per-core dataflow (fp32r = hardware fast-fp32, ~1.5e-4 rel err):
  - x arrives natural-layout (b, k); contraction must sit on SBUF
    partitions, so each 128x128 block is transposed on the tensor engine
    (fp32r transpose mode, ~77ns/tile measured) via an identity matmul,
    staged through PSUM, copied to SBUF by DVE/ACT.
  - loo matmul: stationary = W_embT k-tile (128x128), moving = xT
    (128x512) accumulating over 16 k-tiles into one PSUM bank.
  - full-probe matmul (M=8): plain accumulating matmuls at partition
    base 0 (this walrus rejects fp32r matmuls with dst partition base
    != 0, so 32-col-group packing via tile_position is unavailable).
"""

import sys

for _p in ("/opt/trn_rl_repo",):
    if _p not in sys.path:
        sys.path.append(_p)

import numpy as np
import concourse.bacc as bacc
import concourse.mybir as mybir
import concourse.tile as tile
from concourse.bass_utils import run_bass_kernel_spmd

dt = mybir.dt

B, C, D, H = 4096, 128, 128, 8
NCORES = 8
JPC = C // NCORES  # 16 concept (= k) tiles per core
KPC = JPC * D  # 2048 contraction elements per core
BCHUNK = 512  # batch per PSUM accumulation chunk (fp32 bank limit)
NBC = B // BCHUNK  # 8 batch chunks
NBT = BCHUNK // 128  # 4 b-tiles of 128 per chunk

_nc_cache = None


def _build():
    global _nc_cache
    if _nc_cache is not None:
        return _nc_cache

    nc = bacc.Bacc(
        "TRN2", target_bir_lowering=False, debug=False, num_devices=NCORES
    )
    x_d = nc.dram_tensor("x", (B, KPC), dt.float32r, kind="ExternalInput").ap()
    wl_d = nc.dram_tensor(
        "w_loo_t", (JPC, D, C), dt.float32r, kind="ExternalInput"
    ).ap()
    wf_d = nc.dram_tensor(
        "w_full_t", (JPC, D, H), dt.float32r, kind="ExternalInput"
    ).ap()
    id_d = nc.dram_tensor("ident", (128, 128), dt.float32r, kind="ExternalInput").ap()
    yl_d = nc.dram_tensor("y_loo_t", (C, B), dt.float32, kind="ExternalOutput").ap()
    yf_d = nc.dram_tensor("y_full_t", (H, B), dt.float32, kind="ExternalOutput").ap()

    with tile.TileContext(nc) as tc:
        with (
            tc.tile_pool(name="wpool", bufs=1) as wpool,
            tc.tile_pool(name="xpool", bufs=14) as xpool,
            tc.tile_pool(name="xtpool", bufs=8) as xtpool,
            tc.tile_pool(name="ypool", bufs=2) as ypool,
            tc.tile_pool(name="pst", bufs=3, space="PSUM") as pst,
            tc.tile_pool(name="psl", bufs=2, space="PSUM") as psl,
            tc.tile_pool(name="psf", bufs=2, space="PSUM") as psf,
        ):
            wl = wpool.tile([D, JPC, C], dt.float32r)
            wf = wpool.tile([D, JPC, H], dt.float32r)
            ident = wpool.tile([128, 128], dt.float32r)
            nc.sync.dma_start(ident[:], id_d[:])

            for bc in range(NBC):
                xns = []
                for bt in range(NBT):
                    xn = xpool.tile([128, KPC], dt.float32r, tag="xn")
                    xns.append(xn)
                if bc == 0:
                    # k-chunked, bt-interleaved loads so the first transposes
                    # can start after ~1MB instead of ~4MB
                    ck = 512
                    for c0 in range(0, KPC, ck):
                        for bt in range(NBT):
                            r0 = bt * 128
                            nc.sync.dma_start(
                                xns[bt][:, c0 : c0 + ck],
                                x_d[r0 : r0 + 128, c0 : c0 + ck],
                            )
                        if c0 == 0:
                            nc.sync.dma_start(wf[:], wf_d.rearrange("t d h -> d t h"))
                            nc.sync.dma_start(wl[:], wl_d.rearrange("t d i -> d t i"))
                else:
                    for bt in range(NBT):
                        r0 = (bc * NBT + bt) * 128
                        nc.sync.dma_start(xns[bt][:], x_d[r0 : r0 + 128, :])

                acc_l = psl.tile([C, BCHUNK], dt.float32, tag="accl")
                acc_f = psf.tile([H, BCHUNK], dt.float32, tag="accf")
                for kt in range(JPC):
                    ps_xt = pst.tile([128, BCHUNK], dt.float32r, tag="pst")
                    for bt in range(NBT):
                        nc.tensor.transpose(
                            ps_xt[:, bt * 128 : (bt + 1) * 128],
                            xns[bt][:, kt * 128 : (kt + 1) * 128],
                            ident[:],
                        )
                    xt = xtpool.tile([128, BCHUNK], dt.float32r, tag="xt")
                    # split PSUM->SBUF copy load between DVE and ACT
                    if kt % 2 == 0:
                        nc.vector.tensor_copy(xt[:], ps_xt[:])
                    else:
                        nc.scalar.copy(xt[:], ps_xt[:])

                    # full first: its stationary is 8 cols (trivial LDW),
                    # and the loo matmul's 224ns weight load can then hide
                    # under the full matmul's 512-col stream
                    nc.tensor.matmul(
                        acc_f[:],
                        wf[:, kt, :],
                        xt[:],
                        start=(kt == 0),
                        stop=(kt == JPC - 1),
                    )
                    nc.tensor.matmul(
                        acc_l[:],
                        wl[:, kt, :],
                        xt[:],
                        start=(kt == 0),
                        stop=(kt == JPC - 1),
                    )

                yl_sb = ypool.tile([C, BCHUNK], dt.float32, tag="yl")
                nc.vector.tensor_copy(yl_sb[:], acc_l[:])
                nc.sync.dma_start(yl_d[:, bc * BCHUNK : (bc + 1) * BCHUNK], yl_sb[:])

                yf_sb = ypool.tile([H, BCHUNK], dt.float32, tag="yf")
                nc.vector.tensor_copy(yf_sb[:], acc_f[:])
                nc.sync.dma_start(yf_d[:, bc * BCHUNK : (bc + 1) * BCHUNK], yf_sb[:])

    nc.compile()
    _nc_cache = nc
    return nc


def _embed_loo_weights(W_loo):
    # probe i sees concepts j != i; scatter into (C, C, D) with zero row at j=i
    I = np.arange(C)[:, None]
    J = np.arange(C)[None, :]
    src = np.clip(J - (J > I).astype(np.int64), 0, C - 2)  # (C, C)
    W_emb = np.take_along_axis(W_loo, src[:, :, None], axis=1)  # (C, C, D)
    return W_emb * (J != I)[:, :, None].astype(W_loo.dtype)


def _prep_in_maps(x, W_loo, W_full):
    x = np.asarray(x, dtype=np.float32)
    W_emb = _embed_loo_weights(np.asarray(W_loo, dtype=np.float32))
    W_full = np.asarray(W_full, dtype=np.float32)
    ident = np.eye(128, dtype=np.float32)
    in_maps = []
    for c in range(NCORES):
        jsl = slice(c * JPC, (c + 1) * JPC)
        x_c = np.ascontiguousarray(x[:, jsl, :]).reshape(B, KPC)
        wl_c = np.ascontiguousarray(W_emb[:, jsl, :].transpose(1, 2, 0))  # (t,d,i)
        wf_c = np.ascontiguousarray(W_full[:, jsl, :].transpose(1, 2, 0))  # (t,d,h)
        in_maps.append(
            {"x": x_c, "w_loo_t": wl_c, "w_full_t": wf_c, "ident": ident}
        )
    return in_maps


def _assemble(results, b_loo, b_full):
    y_loo_t = np.zeros((C, B), np.float64)
    y_full_t = np.zeros((H, B), np.float64)
    for r in results:
        y_loo_t += r["y_loo_t"]
        y_full_t += r["y_full_t"]
    y_loo = (y_loo_t.T + np.asarray(b_loo, np.float64)[None, :]).astype(np.float32)
    y_full = (y_full_t.T + np.asarray(b_full, np.float64)[None, :]).astype(np.float32)
    return np.concatenate([y_loo, y_full], axis=1)


def run_spmd(x, W_loo, b_loo, W_full, b_full, trace=False):
    nc = _build()
    in_maps = _prep_in_maps(x, W_loo, W_full)
    res = run_bass_kernel_spmd(
        nc, in_maps, core_ids=list(range(NCORES)), trace=trace
    )
    return _assemble(res.results, b_loo, b_full), res


def kernel(x, W_loo, b_loo, W_full, b_full):
    out, _ = run_spmd(x, W_loo, b_loo, W_full, b_full)
    return out


# revision 14
# speedup vs baseline: 1.1281x; 1.0743x over previous
"""TRN2 Bass kernel for nn_ConceptEmbeddingConceptPred.

Computes y = concat([einsum('bjd,ijd->bi', x, W_emb) + b_loo,
                     einsum('bjd,hjd->bh', x, W_full) + b_full], axis=1)
where W_emb is the leave-one-out scatter-embedding of W_loo (zero diagonal).

Flattened, this is a (4096 x 16384) @ (16384 x 136) GEMM.

Distribution: contraction(k)-parallel over the 8 cores — core c owns
concepts j in [16c, 16c+16) (k-slice of 2048). Each core computes a full
(136, 4096) partial product; partials are summed on the host (cheap:
8 x 2.2 MB), bias added, transposed, concatenated.

Per-core dataflow (fp32r = hardware fast-fp32, ~1.5e-4 rel err):
  - x arrives natural-layout (b, k); contraction must sit on SBUF
    partitions, so each 128x128 block is transposed on the tensor engine
    (fp32r transpose mode, ~77ns/tile measured) via an identity matmul,
    staged through PSUM, copied to SBUF by DVE/ACT.
  - loo matmul: stationary = W_embT k-tile (128x128), moving = xT
    (128x512) accumulating over 16 k-tiles into one PSUM bank.
  - full-probe matmul (M=8): plain accumulating matmuls at partition
    base 0 (this walrus rejects fp32r matmuls with dst partition base
    != 0, so 32-col-group packing via tile_position is unavailable).
"""

import sys

for _p in ("/opt/trn_rl_repo",):
    if _p not in sys.path:
        sys.path.append(_p)

import numpy as np
import concourse.bacc as bacc
import concourse.mybir as mybir
import concourse.tile as tile
from concourse.bass_utils import run_bass_kernel_spmd

dt = mybir.dt

B, C, D, H = 4096, 128, 128, 8
NCORES = 8
JPC = C // NCORES  # 16 concept (= k) tiles per core
KPC = JPC * D  # 2048 contraction elements per core
BCHUNK = 512  # batch per PSUM accumulation chunk (fp32 bank limit)
NBC = B // BCHUNK  # 8 batch chunks
NBT = BCHUNK // 128  # 4 b-tiles of 128 per chunk

_nc_cache = None


def _build():
    global _nc_cache
    if _nc_cache is not None:
        return _nc_cache

    nc = bacc.Bacc(
        "TRN2", target_bir_lowering=False, debug=False, num_devices=NCORES
    )
    x_d = nc.dram_tensor("x", (B, KPC), dt.float32r, kind="ExternalInput").ap()
    wl_d = nc.dram_tensor(
        "w_loo_t", (JPC, D, C), dt.float32r, kind="ExternalInput"
    ).ap()
    wf_d = nc.dram_tensor(
        "w_full_t", (JPC, D, H), dt.float32r, kind="ExternalInput"
    ).ap()
    id_d = nc.dram_tensor("ident", (128, 128), dt.float32r, kind="ExternalInput").ap()
    yl_d = nc.dram_tensor("y_loo_t", (C, B), dt.float32, kind="ExternalOutput").ap()
    yf_d = nc.dram_tensor("y_full_t", (H, B), dt.float32, kind="ExternalOutput").ap()

    with tile.TileContext(nc) as tc:
        with (
            tc.tile_pool(name="wpool", bufs=1) as wpool,
            tc.tile_pool(name="xpool", bufs=14) as xpool,
            tc.tile_pool(name="xtpool", bufs=8) as xtpool,
            tc.tile_pool(name="ypool", bufs=2) as ypool,
            tc.tile_pool(name="pst", bufs=3, space="PSUM") as pst,
            tc.tile_pool(name="psl", bufs=2, space="PSUM") as psl,
            tc.tile_pool(name="psf", bufs=2, space="PSUM") as psf,
        ):
            wl = wpool.tile([D, JPC, C], dt.float32r)
            wf = wpool.tile([D, JPC, H], dt.float32r)
            ident = wpool.tile([128, 128], dt.float32r)
            nc.sync.dma_start(ident[:], id_d[:])

            for bc in range(NBC):
                xns = []
                for bt in range(NBT):
                    xn = xpool.tile([128, KPC], dt.float32r, tag="xn")
                    xns.append(xn)
                if bc == 0:
                    # k-chunked, bt-interleaved loads so the first transposes
                    # can start after ~1MB instead of ~4MB
                    ck = 512
                    for c0 in range(0, KPC, ck):
                        for bt in range(NBT):
                            r0 = bt * 128
                            nc.sync.dma_start(
                                xns[bt][:, c0 : c0 + ck],
                                x_d[r0 : r0 + 128, c0 : c0 + ck],
                            )
                        if c0 == 0:
                            nc.sync.dma_start(wf[:], wf_d.rearrange("t d h -> d t h"))
                            nc.sync.dma_start(wl[:], wl_d.rearrange("t d i -> d t i"))
                else:
                    for bt in range(NBT):
                        r0 = (bc * NBT + bt) * 128
                        eng = nc.sync if bt % 2 == 0 else nc.scalar
                        eng.dma_start(xns[bt][:], x_d[r0 : r0 + 128, :])

                acc_l = psl.tile([C, BCHUNK], dt.float32, tag="accl")
                acc_f = psf.tile([H, BCHUNK], dt.float32, tag="accf")
                for kt in range(JPC):
                    ps_xt = pst.tile([128, BCHUNK], dt.float32r, tag="pst")
                    for bt in range(NBT):
                        nc.tensor.transpose(
                            ps_xt[:, bt * 128 : (bt + 1) * 128],
                            xns[bt][:, kt * 128 : (kt + 1) * 128],
                            ident[:],
                        )
                    xt = xtpool.tile([128, BCHUNK], dt.float32r, tag="xt")
                    # split PSUM->SBUF copy load between DVE and ACT
                    if kt % 2 == 0:
                        nc.vector.tensor_copy(xt[:], ps_xt[:])
                    else:
                        nc.scalar.copy(xt[:], ps_xt[:])

                    # full first: its stationary is 8 cols (trivial LDW),
                    # and the loo matmul's 224ns weight load can then hide
                    # under the full matmul's 512-col stream
                    nc.tensor.matmul(
                        acc_f[:],
                        wf[:, kt, :],
                        xt[:],
                        start=(kt == 0),
                        stop=(kt == JPC - 1),
                    )
                    nc.tensor.matmul(
                        acc_l[:],
                        wl[:, kt, :],
                        xt[:],
                        start=(kt == 0),
                        stop=(kt == JPC - 1),
                    )

                yl_sb = ypool.tile([C, BCHUNK], dt.float32, tag="yl")
                nc.vector.tensor_copy(yl_sb[:], acc_l[:])
                nc.sync.dma_start(yl_d[:, bc * BCHUNK : (bc + 1) * BCHUNK], yl_sb[:])

                yf_sb = ypool.tile([H, BCHUNK], dt.float32, tag="yf")
                nc.vector.tensor_copy(yf_sb[:], acc_f[:])
                nc.sync.dma_start(yf_d[:, bc * BCHUNK : (bc + 1) * BCHUNK], yf_sb[:])

    nc.compile()
    _nc_cache = nc
    return nc


def _embed_loo_weights(W_loo):
    # probe i sees concepts j != i; scatter into (C, C, D) with zero row at j=i
    I = np.arange(C)[:, None]
    J = np.arange(C)[None, :]
    src = np.clip(J - (J > I).astype(np.int64), 0, C - 2)  # (C, C)
    W_emb = np.take_along_axis(W_loo, src[:, :, None], axis=1)  # (C, C, D)
    return W_emb * (J != I)[:, :, None].astype(W_loo.dtype)


def _prep_in_maps(x, W_loo, W_full):
    x = np.asarray(x, dtype=np.float32)
    W_emb = _embed_loo_weights(np.asarray(W_loo, dtype=np.float32))
    W_full = np.asarray(W_full, dtype=np.float32)
    ident = np.eye(128, dtype=np.float32)
    in_maps = []
    for c in range(NCORES):
        jsl = slice(c * JPC, (c + 1) * JPC)
        x_c = np.ascontiguousarray(x[:, jsl, :]).reshape(B, KPC)
        wl_c = np.ascontiguousarray(W_emb[:, jsl, :].transpose(1, 2, 0))  # (t,d,i)
        wf_c = np.ascontiguousarray(W_full[:, jsl, :].transpose(1, 2, 0))  # (t,d,h)
        in_maps.append(
            {"x": x_c, "w_loo_t": wl_c, "w_full_t": wf_c, "ident": ident}
        )
    return in_maps


def _assemble(results, b_loo, b_full):
    y_loo_t = np.zeros((C, B), np.float64)
    y_full_t = np.zeros((H, B), np.float64)
    for r in results:
        y_loo_t += r["y_loo_t"]
        y_full_t += r["y_full_t"]
    y_loo = (y_loo_t.T + np.asarray(b_loo, np.float64)[None, :]).astype(np.float32)
    y_full = (y_full_t.T + np.asarray(b_full, np.float64)[None, :]).astype(np.float32)
    return np.concatenate([y_loo, y_full], axis=1)


def run_spmd(x, W_loo, b_loo, W_full, b_full, trace=False):
    nc = _build()
    in_maps = _prep_in_maps(x, W_loo, W_full)
    res = run_bass_kernel_spmd(
        nc, in_maps, core_ids=list(range(NCORES)), trace=trace
    )
    return _assemble(res.results, b_loo, b_full), res


def kernel(x, W_loo, b_loo, W_full, b_full):
    out, _ = run_spmd(x, W_loo, b_loo, W_full, b_full)
    return out
